# revision 4
# baseline (speedup 1.0000x reference)
"""Trainium2 Bass kernel for nn_DifferentiableCBFLayer.

Batched QP safety filter: per-sample constraint build (G/h) + 100 ADMM
iterations, 65536 samples. Data-parallel across 8 NeuronCores (8192
samples/core), laid out as [128 partitions x 64 groups] per core.

Restructured ADMM (validated vs reference, rel err ~5e-6):
    precompute  A = [a1 a2 a3] (37x3, a3 + box rows constant),
                M = Q + A^T A,  Minv via adjugate,
                B3_j = (Minv A^T)_j  (+ col 38 = c_j = -(Minv q)_j),
                b (rhs), t0 = min(0, b), y0 = 0
    iterate     x_j = sum_k B3ext_j[k] * text[k]        (text = [t, 1])
                w   = a1*x1 + a2*x2 + y   (- x3 on obs rows & row 36)
                z   = min(w, b)
                t   = 2z - w ;  y = w - z
    output      u_safe = (x1, x2)

Hardware note: scalar_tensor_tensor (STT struct) carries only ONE sync-wait
slot, so every STT input must be DVE-produced (never a fresh DMA tile) —
inputs are first repacked via tensor_copy, which absorbs the DMA waits.
"""

import numpy as np

_B_FULL = 65536
_N_CORES = 8
_BC = _B_FULL // _N_CORES     # 8192 samples per core
_P = 128                      # SBUF partitions
_C = _BC // _P                # 64 groups per partition
_NO = 16                      # obstacle rows
_NA = 8                       # agent rows
_M = 37                       # constraint rows (16 obs + 8 avoid + 8 conn + 5 box)
_ME = 38                      # + homogeneous col for c_j
_N_ITERS = 100
_M33 = 2.0 * 100.0 + 17.0     # Q_33 + sum(a3^2) = 200 + 17, constant

_cache = {}


def _build_program():
    import concourse.bass as bass
    import concourse.tile as tile
    from concourse import mybir

    Alu = mybir.AluOpType
    f32 = mybir.dt.float32
    nc = bass.Bass()

    ins = {
        "u_nominal": nc.declare_dram_parameter("u_nominal", [_BC, 2], f32, isOutput=False),
        "v_current": nc.declare_dram_parameter("v_current", [_BC, 1], f32, isOutput=False),
        "p_obs": nc.declare_dram_parameter("p_obs", [_BC, _NO, 2], f32, isOutput=False),
        "p_agents": nc.declare_dram_parameter("p_agents", [_BC, _NA, 2], f32, isOutput=False),
        "v_agents_local": nc.declare_dram_parameter("v_agents_local", [_BC, _NA, 2], f32, isOutput=False),
        "agent_active": nc.declare_dram_parameter("agent_active", [_BC, _NA], f32, isOutput=False),
        "obs_active": nc.declare_dram_parameter("obs_active", [_BC, _NO], f32, isOutput=False),
    }
    out_ext = nc.declare_dram_parameter("out", [_BC, 2], f32, isOutput=True)

    with tile.TileContext(nc) as tc:
        with tc.tile_pool(name="main", bufs=1) as pool:
            vec = nc.vector

            def tt(out, in0, in1, op):
                vec.tensor_tensor(out=out, in0=in0, in1=in1, op=op)

            def stt(out, in0, s, op0, in1, op1):
                vec.scalar_tensor_tensor(out=out, in0=in0, scalar=s, in1=in1, op0=op0, op1=op1)

            def ts(out, in0, s1, op0, s2=None, op1=Alu.bypass):
                vec.tensor_scalar(out=out, in0=in0, scalar1=s1, scalar2=s2, op0=op0, op1=op1)

            def bc(ap2d, n):
                # [128, C] -> [128, C, n] stride-0 broadcast view
                return ap2d.unsqueeze(2).broadcast_to([_P, _C, n])

            # ---------------- input tiles + DMA ----------------
            t_u = pool.tile([_P, _C, 2], f32, name="t_u")
            t_v = pool.tile([_P, _C, 1], f32, name="t_v")
            t_po = pool.tile([_P, _C, _NO, 2], f32, name="t_po")
            t_pa = pool.tile([_P, _C, _NA, 2], f32, name="t_pa")
            t_va = pool.tile([_P, _C, _NA, 2], f32, name="t_va")
            t_aa = pool.tile([_P, _C, _NA], f32, name="t_aa")
            t_oa = pool.tile([_P, _C, _NO], f32, name="t_oa")

            nc.sync.dma_start(out=t_u[:], in_=ins["u_nominal"].rearrange("(p c) k -> p c k", p=_P))
            nc.sync.dma_start(out=t_v[:], in_=ins["v_current"].rearrange("(p c) k -> p c k", p=_P))
            nc.sync.dma_start(out=t_po[:], in_=ins["p_obs"].rearrange("(p c) n k -> p c n k", p=_P))
            nc.sync.dma_start(out=t_pa[:], in_=ins["p_agents"].rearrange("(p c) n k -> p c n k", p=_P))
            nc.sync.dma_start(out=t_va[:], in_=ins["v_agents_local"].rearrange("(p c) n k -> p c n k", p=_P))
            nc.sync.dma_start(out=t_aa[:], in_=ins["agent_active"].rearrange("(p c) n -> p c n", p=_P))
            nc.sync.dma_start(out=t_oa[:], in_=ins["obs_active"].rearrange("(p c) n -> p c n", p=_P))

            # packed field copies (DVE-produced; absorb all DMA waits)
            lx = pool.tile([_P, _C, _NO], f32, name="lx")
            ly = pool.tile([_P, _C, _NO], f32, name="ly")
            oa = pool.tile([_P, _C, _NO], f32, name="oa")
            lxa = pool.tile([_P, _C, _NA], f32, name="lxa")
            lya = pool.tile([_P, _C, _NA], f32, name="lya")
            vjx = pool.tile([_P, _C, _NA], f32, name="vjx")
            vjy = pool.tile([_P, _C, _NA], f32, name="vjy")
            aa = pool.tile([_P, _C, _NA], f32, name="aa")
            vt = pool.tile([_P, _C, 1], f32, name="vt")
            ut = pool.tile([_P, _C, 2], f32, name="ut")

            vec.tensor_copy(out=lx[:], in_=t_po[:, :, :, 0])
            vec.tensor_copy(out=ly[:], in_=t_po[:, :, :, 1])
            vec.tensor_copy(out=oa[:], in_=t_oa[:])
            vec.tensor_copy(out=lxa[:], in_=t_pa[:, :, :, 0])
            vec.tensor_copy(out=lya[:], in_=t_pa[:, :, :, 1])
            vec.tensor_copy(out=vjx[:], in_=t_va[:, :, :, 0])
            vec.tensor_copy(out=vjy[:], in_=t_va[:, :, :, 1])
            vec.tensor_copy(out=aa[:], in_=t_aa[:])
            vec.tensor_copy(out=vt[:], in_=t_v[:])
            vec.tensor_copy(out=ut[:], in_=t_u[:])

            # ---------------- persistent state ----------------
            a1 = pool.tile([_P, _C, _M], f32, name="a1")
            a2 = pool.tile([_P, _C, _M], f32, name="a2")
            b = pool.tile([_P, _C, _M], f32, name="b")
            B3 = [pool.tile([_P, _C, _ME], f32, name=f"B3_{j}") for j in range(3)]
            text = pool.tile([_P, _C, _ME], f32, name="text")
            y = pool.tile([_P, _C, _M], f32, name="y")

            # scratch (aliased aggressively; all reuse is same-engine serial)
            m1 = pool.tile([_P, _C, _ME], f32, name="m1")
            m2 = pool.tile([_P, _C, _ME], f32, name="m2")
            vz = pool.tile([_P, _C, _M], f32, name="vz")   # v1, then z
            ww = pool.tile([_P, _C, _M], f32, name="ww")   # s, then w
            x1 = pool.tile([_P, _C], f32, name="x1")
            x2 = pool.tile([_P, _C], f32, name="x2")
            x3 = pool.tile([_P, _C], f32, name="x3")
            s1 = pool.tile([_P, _C], f32, name="s1")
            s2 = pool.tile([_P, _C], f32, name="s2")
            o_t = pool.tile([_P, _C, 2], f32, name="o_t")
            Mv = [pool.tile([_P, _C], f32, name=f"Mv{i}") for i in range(5)]  # M11,M12,M13,M22,M23
            Cf = [pool.tile([_P, _C], f32, name=f"Cf{i}") for i in range(6)]  # c11,c12,c13,c22,c23,c33

            v64 = vt[:, :, 0]                       # [128, C]
            bv16 = vt.broadcast_to([_P, _C, _NO])
            bv8 = vt.broadcast_to([_P, _C, _NA])

            # ---------------- build a1, a2, b ----------------
            # obstacle rows 0:16
            q1, q2, q3, q4 = m1[:, :, 0:_NO], m2[:, :, 0:_NO], vz[:, :, 0:_NO], ww[:, :, 0:_NO]
            ts(a1[:, :, 0:_NO], lx, 2.0, Alu.mult)
            stt(a2[:, :, 0:_NO], ly, 2.0, Alu.mult, bv16, Alu.mult)
            tt(q1, lx, lx, Alu.mult)
            tt(q2, ly, ly, Alu.mult)
            tt(q3, q1, q2, Alu.add)                      # lx^2+ly^2
            stt(q4, lx, -4.0, Alu.mult, bv16, Alu.mult)  # -4 lx v
            tt(q3, q3, q4, Alu.add)
            tt(s1, v64, v64, Alu.mult)                   # v^2
            ts(s2, s1, 2.0, Alu.mult, -0.25, Alu.add)    # 2v^2 - 0.25
            tt(q3, q3, bc(s2, _NO), Alu.add)
            tt(b[:, :, 0:_NO], q3, oa, Alu.mult)

            # agent rows 16:24 (avoid), 24:32 (conn)
            g1, g2, g3, g4, g5 = (m1[:, :, 0:_NA], m2[:, :, 0:_NA], vz[:, :, 0:_NA],
                                  ww[:, :, 0:_NA], m1[:, :, 8:16])
            stt(a1[:, :, 16:24], lxa, 2.0, Alu.mult, aa, Alu.mult)
            stt(a1[:, :, 24:32], lxa, -2.0, Alu.mult, aa, Alu.mult)
            tt(g1, bv8, vjx, Alu.subtract)               # v - vjx
            tt(g2, lya, g1, Alu.mult)
            tt(g3, lxa, vjy, Alu.mult)
            tt(g2, g2, g3, Alu.add)                      # Gw/2 = ly(v-vjx)+lx vjy
            stt(a2[:, :, 16:24], g2, 2.0, Alu.mult, aa, Alu.mult)
            stt(a2[:, :, 24:32], g2, -2.0, Alu.mult, aa, Alu.mult)
            # SP = 2v^2 - 4 v vjx + 2(vjx^2+vjy^2) - 4 lx v + 4 lx vjx + 4 ly vjy + lx^2 + ly^2
            tt(g1, vjx, vjx, Alu.mult)
            tt(g2, vjy, vjy, Alu.mult)
            tt(g1, g1, g2, Alu.add)                      # vjx^2+vjy^2
            tt(g2, lxa, lxa, Alu.mult)
            tt(g3, lya, lya, Alu.mult)
            tt(g2, g2, g3, Alu.add)                      # lx^2+ly^2
            stt(g4, g1, 2.0, Alu.mult, g2, Alu.add)      # acc
            tt(g1, bv8, vjx, Alu.mult)
            stt(g4, g1, -4.0, Alu.mult, g4, Alu.add)
            tt(g1, lxa, bv8, Alu.mult)
            stt(g4, g1, -4.0, Alu.mult, g4, Alu.add)
            tt(g1, lxa, vjx, Alu.mult)
            stt(g4, g1, 4.0, Alu.mult, g4, Alu.add)
            tt(g1, lya, vjy, Alu.mult)
            stt(g4, g1, 4.0, Alu.mult, g4, Alu.add)
            ts(s2, s1, 2.0, Alu.mult)                    # 2v^2
            tt(g4, g4, bc(s2, _NA), Alu.add)             # SP
            stt(g5, g4, -0.25, Alu.add, aa, Alu.mult)
            vec.tensor_copy(out=b[:, :, 16:24], in_=g5)
            ts(g5, g4, -1.0, Alu.mult, 100.0, Alu.add)
            tt(b[:, :, 24:32], g5, aa, Alu.mult)

            # box rows 32:37
            vec.memset(a1[:, :, 32:37], 0.0)
            vec.memset(a2[:, :, 32:37], 0.0)
            vec.memset(a1[:, :, 32], -1.0)
            vec.memset(a1[:, :, 33], 1.0)
            vec.memset(a2[:, :, 34], -1.0)
            vec.memset(a2[:, :, 35], 1.0)
            vec.memset(b[:, :, 32:36], 1.0)
            vec.memset(b[:, :, 36], 0.0)

            # ---------------- M = Q + A^T A, Minv, B3, c ----------------
            w37 = m1[:, :, 0:_M]
            tt(w37, a1, a1, Alu.mult)
            vec.reduce_sum(out=Mv[0], in_=w37, axis=mybir.AxisListType.X)   # sum a1^2 (box adds 2)
            tt(w37, a1, a2, Alu.mult)
            vec.reduce_sum(out=Mv[1], in_=w37, axis=mybir.AxisListType.X)   # M12
            tt(w37, a2, a2, Alu.mult)
            vec.reduce_sum(out=Mv[3], in_=w37, axis=mybir.AxisListType.X)
            vec.reduce_sum(out=s1, in_=a1[:, :, 0:_NO], axis=mybir.AxisListType.X)
            ts(Mv[2], s1, -1.0, Alu.mult)                                   # M13
            vec.reduce_sum(out=s1, in_=a2[:, :, 0:_NO], axis=mybir.AxisListType.X)
            ts(Mv[4], s1, -1.0, Alu.mult)                                   # M23
            ts(Mv[0], Mv[0], 2.0, Alu.add)                                  # M11
            ts(Mv[3], Mv[3], 2.0, Alu.add)                                  # M22
            M11, M12, M13, M22, M23 = Mv
            # cofactors (M33 const)
            tt(s1, M23, M23, Alu.mult)
            stt(Cf[0], M22, _M33, Alu.mult, s1, Alu.subtract)               # c11
            tt(s1, M13, M23, Alu.mult)
            stt(Cf[1], M12, -_M33, Alu.mult, s1, Alu.add)                   # c12
            tt(s1, M12, M23, Alu.mult)
            tt(s2, M13, M22, Alu.mult)
            tt(Cf[2], s1, s2, Alu.subtract)                                 # c13
            tt(s1, M13, M13, Alu.mult)
            stt(Cf[3], M11, _M33, Alu.mult, s1, Alu.subtract)               # c22
            tt(s1, M12, M13, Alu.mult)
            tt(s2, M11, M23, Alu.mult)
            tt(Cf[4], s1, s2, Alu.subtract)                                 # c23
            tt(s1, M11, M22, Alu.mult)
            tt(s2, M12, M12, Alu.mult)
            tt(Cf[5], s1, s2, Alu.subtract)                                 # c33
            # det, 1/det, scale cofactors
            tt(s1, M11, Cf[0], Alu.mult)
            tt(s2, M12, Cf[1], Alu.mult)
            tt(s1, s1, s2, Alu.add)
            tt(s2, M13, Cf[2], Alu.mult)
            tt(s1, s1, s2, Alu.add)
            vec.reciprocal(out=s2, in_=s1)
            for i in range(6):
                tt(Cf[i], Cf[i], s2, Alu.mult)
            # B3_j = Minv_j. @ A^T ; col 37 = c_j = 2(Minv_j1 u1 + Minv_j2 u2)
            rows = [(Cf[0], Cf[1], Cf[2]), (Cf[1], Cf[3], Cf[4]), (Cf[2], Cf[4], Cf[5])]
            u1 = ut[:, :, 0]
            u2 = ut[:, :, 1]
            for j in range(3):
                cj1, cj2, cj3 = rows[j]
                Bj = B3[j][:, :, 0:_M]
                tt(Bj, a1, bc(cj1, _M), Alu.mult)
                tt(w37, a2, bc(cj2, _M), Alu.mult)
                tt(Bj, Bj, w37, Alu.add)
                tt(B3[j][:, :, 0:_NO], B3[j][:, :, 0:_NO], bc(cj3, _NO), Alu.subtract)
                tt(B3[j][:, :, 36], B3[j][:, :, 36], cj3, Alu.subtract)
                tt(s1, cj1, u1, Alu.mult)
                tt(s2, cj2, u2, Alu.mult)
                tt(s1, s1, s2, Alu.add)
                ts(B3[j][:, :, 37], s1, 2.0, Alu.mult)

            # ---------------- ADMM state init ----------------
            vec.memset(text[:, :, 37], 1.0)
            vec.tensor_scalar_min(out=text[:, :, 0:_M], in0=b, scalar1=0.0)  # t0 = min(0, b)
            vec.memset(y[:], 0.0)

            # ---------------- 100 ADMM iterations ----------------
            for it in range(_N_ITERS):
                tt(m1[:], B3[0], text, Alu.mult)
                vec.reduce_sum(out=x1, in_=m1[:], axis=mybir.AxisListType.X)
                tt(m2[:], B3[1], text, Alu.mult)
                vec.reduce_sum(out=x2, in_=m2[:], axis=mybir.AxisListType.X)
                if it == _N_ITERS - 1:
                    break
                tt(m1[:], B3[2], text, Alu.mult)
                vec.reduce_sum(out=x3, in_=m1[:], axis=mybir.AxisListType.X)
                tt(vz[:], a1, bc(x1, _M), Alu.mult)            # v1
                tt(m2[:, :, 0:_M], a2, bc(x2, _M), Alu.mult)   # v2
                tt(ww[:], vz, m2[:, :, 0:_M], Alu.add)         # s = v1+v2
                tt(ww[:], ww, y, Alu.add)                      # w = s + y
                tt(ww[:, :, 0:_NO], ww[:, :, 0:_NO], bc(x3, _NO), Alu.subtract)
                tt(ww[:, :, 36], ww[:, :, 36], x3, Alu.subtract)
                tt(vz[:], ww, b, Alu.min)                      # z = min(w, b)
                stt(text[:, :, 0:_M], vz, 2.0, Alu.mult, ww, Alu.subtract)   # t = 2z - w
                tt(y[:], ww, vz, Alu.subtract)                 # y = w - z

            # ---------------- output ----------------
            vec.tensor_copy(out=o_t[:, :, 0], in_=x1)
            vec.tensor_copy(out=o_t[:, :, 1], in_=x2)
            nc.sync.dma_start(out=out_ext.rearrange("(p c) k -> p c k", p=_P), in_=o_t[:])

    _split_excess_waits(nc, mybir)
    return nc


def _split_excess_waits(nc, mybir):
    """Walrus ISA structs carry a limited number of sync-wait slots (1 for
    STT/CTRL structs, 2 for most compute structs); the Tile scheduler can
    attach more (e.g. the tail drain waits on every DMA queue sem).  Move
    excess waits onto same-engine single-wait NoOps inserted directly
    before the instruction."""
    def limit_for(inst):
        nm = type(inst).__name__
        if nm in ("InstDrain", "InstNoOp", "InstTensorScalarPtr"):
            return 1
        return 2

    for fn in nc.m.functions:
        for blk in fn.blocks:
            il = list(blk.instructions)
            new, changed = [], False
            for inst in il:
                si = inst.sync_info
                lim = limit_for(inst)
                if si is not None and len(si.on_wait) > lim:
                    waits = list(si.on_wait)
                    k = 0
                    while len(waits) > lim:
                        new.append(mybir.InstNoOp(
                            name=f"{inst.name}-waitsplit{k}",
                            ins=[], outs=[], engine=inst.engine,
                            sync_info=mybir.SyncInfo(on_wait=[waits.pop(0)], on_update=[]),
                            bass_nofuse=True,
                        ))
                        k += 1
                    inst.sync_info = mybir.SyncInfo(on_wait=waits, on_update=si.on_update)
                    changed = True
                new.append(inst)
            if changed:
                blk.instructions = new


def _get_program():
    if "nc" not in _cache:
        _cache["nc"] = _build_program()
    return _cache["nc"]


def _run(in_maps, trace=False):
    from concourse.bass_utils import run_bass_kernel_spmd

    nc = _get_program()
    return run_bass_kernel_spmd(nc, in_maps, list(range(_N_CORES)), trace=trace)


def _shard(inputs):
    in_maps = []
    for i in range(_N_CORES):
        sl = slice(i * _BC, (i + 1) * _BC)
        in_maps.append({
            k: np.ascontiguousarray(np.asarray(v)[sl], dtype=np.float32)
            for k, v in inputs.items()
        })
    return in_maps


def kernel(**inputs):
    res = _run(_shard(inputs))
    return np.concatenate([r["out"] for r in res.results], axis=0)


# revision 31
# speedup vs baseline: 23.4937x; 23.4937x over previous
"""Trainium2 Bass kernel for nn_DifferentiableCBFLayer.

Batched QP safety filter: per-sample constraint build (G/h) + 100 ADMM
iterations, 65536 samples. Data-parallel across 8 NeuronCores (8192
samples/core), laid out as [128 partitions x 64 groups] per core.

Restructured ADMM (validated vs reference, rel err ~5e-6):
    precompute  A = [a1 a2 a3] (37x3, a3 + box rows constant),
                M = Q + A^T A,  Minv via adjugate,
                B3_j = (Minv A^T)_j  (+ col 38 = c_j = -(Minv q)_j),
                b (rhs), t0 = min(0, b), y0 = 0
    iterate     x_j = sum_k B3ext_j[k] * text[k]        (text = [t, 1])
                w   = a1*x1 + a2*x2 + y   (- x3 on obs rows & row 36)
                z   = min(w, b)
                t   = 2z - w ;  y = w - z
    output      u_safe = (x1, x2)

Hardware note: scalar_tensor_tensor (STT struct) carries only ONE sync-wait
slot, so every STT input must be DVE-produced (never a fresh DMA tile) —
inputs are first repacked via tensor_copy, which absorbs the DMA waits.
"""

import numpy as np

_B_FULL = 65536
_N_CORES = 8
_BC = _B_FULL // _N_CORES     # 8192 samples per core
_P = 128                      # SBUF partitions
_C = _BC // _P                # 64 groups per partition
_NO = 16                      # obstacle rows
_NA = 8                       # agent rows
_M = 37                       # constraint rows (16 obs + 8 avoid + 8 conn + 5 box)
_ME = 38                      # + homogeneous col for c_j
_N_ITERS = 100
_M33 = 2.0 * 100.0 + 17.0     # Q_33 + sum(a3^2) = 200 + 17, constant

_cache = {}

_SEGSUM_NAME = "SEGSUM_MULT_ANT"


def _register_segsum_op():
    """Custom DVE op: per-row segmented inclusive scan of Src0*Src1 along the
    innermost free dim of a [P, S, N] AP.  out[p, s, n] = sum_{k<=n} in0*in1.
    Element N-1 of each row is the row's dot product — this fuses the
    mult + tensor_reduce pair of the ADMM x-step into ONE DVE pass.

    Built from the stock Scan lowering (seed + steady) plus a hand-added
    `step` uop that fires on SUB_DIM_DONE and re-seeds the scan feedback
    from the Zero delay-lane for the first element of each new row — the
    same FSM shape the PageIdx ops use, with a reset instead of an
    increment."""
    import copy as _copy
    from concourse import dve_ops as _dops
    from concourse.dve_spec import Spec, Scan, Src0, Src1, AluOp, lower
    from concourse.dve_uop import DveOpSpec, Trigger, AluInp

    if _SEGSUM_NAME in _dops._SUB_OPCODE_FOR_NAME:
        return next(op for op in _dops.OPS if op.name == _SEGSUM_NAME)

    def _ref(in0, in1, c0, c1, c2):
        # in0 carries the [P, S, N] subdim structure; in1/out may be flat
        assert in0.ndim == 3, f"segsum expects [P,S,N] in0, got {in0.shape}"
        a = in0.astype(np.float32)
        bb = np.asarray(in1, np.float32).reshape(a.shape)
        return np.cumsum(a * bb, axis=-1, dtype=np.float32)

    spec = Spec(body=Scan(AluOp.ADD, Src0 * Src1), reference=_ref)
    row = _dops._CUSTOM_DVE_ROW_BASE + len(_dops.OPS)
    assert row < 0x20

    class _SegsumOp:
        name = _SEGSUM_NAME
        subdim = True

        def __init__(self):
            self.spec = spec
            self._compiled = {}

        def compile(self, ver):
            if ver in self._compiled:
                return self._compiled[ver]
            uops = lower(self.spec, ver=ver)
            assert len(uops) == 2, f"expected seed+steady uops, got {len(uops)}"
            seed, steady = uops
            step = _copy.deepcopy(steady)
            # dp[1] is the scan-combine stage: ADD(CURR_ALU_OUT, product).
            # For the first element of a new row, read the Zero lane instead
            # of the scan feedback (same lane the seed uop uses).
            assert steady.datapath_config[1].alu_src0 == AluInp.CURR_ALU_OUT
            step.datapath_config[1].alu_src0 = AluInp.PREV_DELAY_2
            step.trigger = (Trigger.SRC_TENSOR_DONE, Trigger.SUB_DIM_DONE,
                            Trigger.COUNT)
            step.repeat_count = 1
            step.next_uop = (0, 2, 1)
            steady.trigger = (Trigger.SRC_TENSOR_DONE, Trigger.SUB_DIM_DONE,
                              Trigger.NONE)
            steady.next_uop = (0, 2, 0)
            r = DveOpSpec(name=self.name, opcode=row,
                          uops=[seed, steady, step], rd1_en=True)
            self._compiled[ver] = r
            return r

    op = _SegsumOp()
    _dops.OPS.append(op)
    _dops._SUB_OPCODE_FOR_NAME[_SEGSUM_NAME] = row
    _dops.CUSTOM_DVE_SPECS[_SEGSUM_NAME] = spec
    return op


# segsum: fused mult+segmented-scan custom DVE op — validated in CoreSim but
# this container's walrus build rejects ALL InstCustomDveAnt encodings
# ("ISA wrong length" even for stock production ops), so default off.
def _build_program(split_waits=True, n_iters=_N_ITERS, segsum=False):
    import concourse.bass as bass
    import concourse.tile as tile
    from concourse import mybir

    Alu = mybir.AluOpType
    f32 = mybir.dt.float32
    nc = bass.Bass()

    ins = {
        "u_nominal": nc.declare_dram_parameter("u_nominal", [_BC, 2], f32, isOutput=False),
        "v_current": nc.declare_dram_parameter("v_current", [_BC, 1], f32, isOutput=False),
        "p_obs": nc.declare_dram_parameter("p_obs", [_BC, _NO, 2], f32, isOutput=False),
        "p_agents": nc.declare_dram_parameter("p_agents", [_BC, _NA, 2], f32, isOutput=False),
        "v_agents_local": nc.declare_dram_parameter("v_agents_local", [_BC, _NA, 2], f32, isOutput=False),
        "agent_active": nc.declare_dram_parameter("agent_active", [_BC, _NA], f32, isOutput=False),
        "obs_active": nc.declare_dram_parameter("obs_active", [_BC, _NO], f32, isOutput=False),
    }
    out_ext = nc.declare_dram_parameter("out", [_BC, 2], f32, isOutput=True)

    with tile.TileContext(nc) as tc:
        with tc.tile_pool(name="main", bufs=1) as pool:
            vec = nc.vector

            def tt(out, in0, in1, op):
                vec.tensor_tensor(out=out, in0=in0, in1=in1, op=op)

            def stt(out, in0, s, op0, in1, op1):
                vec.scalar_tensor_tensor(out=out, in0=in0, scalar=s, in1=in1, op0=op0, op1=op1)

            def ts(out, in0, s1, op0, s2=None, op1=Alu.bypass):
                vec.tensor_scalar(out=out, in0=in0, scalar1=s1, scalar2=s2, op0=op0, op1=op1)

            def bc(ap2d, n):
                # [128, C] -> [128, C, n] stride-0 broadcast view
                return ap2d.unsqueeze(2).broadcast_to([_P, _C, n])

            # ---------------- input tiles + DMA ----------------
            t_u = pool.tile([_P, _C, 2], f32, name="t_u")
            t_v = pool.tile([_P, _C, 1], f32, name="t_v")
            t_po = pool.tile([_P, _C, _NO, 2], f32, name="t_po")
            t_pa = pool.tile([_P, _C, _NA, 2], f32, name="t_pa")
            t_va = pool.tile([_P, _C, _NA, 2], f32, name="t_va")
            t_aa = pool.tile([_P, _C, _NA], f32, name="t_aa")
            t_oa = pool.tile([_P, _C, _NO], f32, name="t_oa")

            nc.sync.dma_start(out=t_u[:], in_=ins["u_nominal"].rearrange("(p c) k -> p c k", p=_P))
            nc.sync.dma_start(out=t_v[:], in_=ins["v_current"].rearrange("(p c) k -> p c k", p=_P))
            nc.sync.dma_start(out=t_po[:], in_=ins["p_obs"].rearrange("(p c) n k -> p c n k", p=_P))
            nc.sync.dma_start(out=t_pa[:], in_=ins["p_agents"].rearrange("(p c) n k -> p c n k", p=_P))
            nc.sync.dma_start(out=t_va[:], in_=ins["v_agents_local"].rearrange("(p c) n k -> p c n k", p=_P))
            nc.sync.dma_start(out=t_aa[:], in_=ins["agent_active"].rearrange("(p c) n -> p c n", p=_P))
            nc.sync.dma_start(out=t_oa[:], in_=ins["obs_active"].rearrange("(p c) n -> p c n", p=_P))

            # packed field copies (DVE-produced; absorb all DMA waits)
            lx = pool.tile([_P, _C, _NO], f32, name="lx")
            ly = pool.tile([_P, _C, _NO], f32, name="ly")
            oa = pool.tile([_P, _C, _NO], f32, name="oa")
            lxa = pool.tile([_P, _C, _NA], f32, name="lxa")
            lya = pool.tile([_P, _C, _NA], f32, name="lya")
            vjx = pool.tile([_P, _C, _NA], f32, name="vjx")
            vjy = pool.tile([_P, _C, _NA], f32, name="vjy")
            aa = pool.tile([_P, _C, _NA], f32, name="aa")
            vt = pool.tile([_P, _C, 1], f32, name="vt")
            ut = pool.tile([_P, _C, 2], f32, name="ut")

            vec.tensor_copy(out=lx[:], in_=t_po[:, :, :, 0])
            vec.tensor_copy(out=ly[:], in_=t_po[:, :, :, 1])
            vec.tensor_copy(out=oa[:], in_=t_oa[:])
            vec.tensor_copy(out=lxa[:], in_=t_pa[:, :, :, 0])
            vec.tensor_copy(out=lya[:], in_=t_pa[:, :, :, 1])
            vec.tensor_copy(out=vjx[:], in_=t_va[:, :, :, 0])
            vec.tensor_copy(out=vjy[:], in_=t_va[:, :, :, 1])
            vec.tensor_copy(out=aa[:], in_=t_aa[:])
            vec.tensor_copy(out=vt[:], in_=t_v[:])
            vec.tensor_copy(out=ut[:], in_=t_u[:])

            # ---------------- persistent state ----------------
            a1 = pool.tile([_P, _C, _M], f32, name="a1")
            a2 = pool.tile([_P, _C, _M], f32, name="a2")
            b = pool.tile([_P, _C, _M], f32, name="b")
            B3 = [pool.tile([_P, _C, _ME], f32, name=f"B3_{j}") for j in range(3)]
            text = pool.tile([_P, _C, _ME], f32, name="text")
            y = pool.tile([_P, _C, _M], f32, name="y")

            # scratch (aliased aggressively; all reuse is same-engine serial)
            mS = [pool.tile([_P, _C, _ME], f32, name=f"mS_{j}") for j in range(3)]
            m1 = mS[0][:]
            m2 = mS[1][:]
            vz = pool.tile([_P, _C, _M], f32, name="vz")   # v1, then e = w - b
            ww = pool.tile([_P, _C, _M], f32, name="ww")   # s, then w
            if segsum:
                # x_j = last scan element of each 38-row of mS[j]
                x1 = mS[0][:, :, _ME - 1]
                x2 = mS[1][:, :, _ME - 1]
                x3 = mS[2][:, :, _ME - 1]
            else:
                x_all = pool.tile([_P, _C, 3], f32, name="x_all")
                x1 = x_all[:, :, 0]
                x2 = x_all[:, :, 1]
                x3 = x_all[:, :, 2]
            s1 = pool.tile([_P, _C], f32, name="s1")
            s2 = pool.tile([_P, _C], f32, name="s2")
            o_t = pool.tile([_P, _C, 2], f32, name="o_t")
            Mv = [pool.tile([_P, _C], f32, name=f"Mv{i}") for i in range(5)]  # M11,M12,M13,M22,M23
            Cf = [pool.tile([_P, _C], f32, name=f"Cf{i}") for i in range(6)]  # c11,c12,c13,c22,c23,c33

            v64 = vt[:, :, 0]                       # [128, C]
            bv16 = vt.broadcast_to([_P, _C, _NO])
            bv8 = vt.broadcast_to([_P, _C, _NA])

            # ---------------- build a1, a2, b ----------------
            # obstacle rows 0:16
            q1, q2, q3, q4 = m1[:, :, 0:_NO], m2[:, :, 0:_NO], vz[:, :, 0:_NO], ww[:, :, 0:_NO]
            ts(a1[:, :, 0:_NO], lx, 2.0, Alu.mult)
            stt(a2[:, :, 0:_NO], ly, 2.0, Alu.mult, bv16, Alu.mult)
            tt(q1, lx, lx, Alu.mult)
            tt(q2, ly, ly, Alu.mult)
            tt(q3, q1, q2, Alu.add)                      # lx^2+ly^2
            stt(q4, lx, -4.0, Alu.mult, bv16, Alu.mult)  # -4 lx v
            tt(q3, q3, q4, Alu.add)
            tt(s1, v64, v64, Alu.mult)                   # v^2
            ts(s2, s1, 2.0, Alu.mult, -0.25, Alu.add)    # 2v^2 - 0.25
            tt(q3, q3, bc(s2, _NO), Alu.add)
            tt(b[:, :, 0:_NO], q3, oa, Alu.mult)

            # agent rows 16:24 (avoid), 24:32 (conn)
            g1, g2, g3, g4, g5 = (m1[:, :, 0:_NA], m2[:, :, 0:_NA], vz[:, :, 0:_NA],
                                  ww[:, :, 0:_NA], m1[:, :, 8:16])
            stt(a1[:, :, 16:24], lxa, 2.0, Alu.mult, aa, Alu.mult)
            stt(a1[:, :, 24:32], lxa, -2.0, Alu.mult, aa, Alu.mult)
            tt(g1, bv8, vjx, Alu.subtract)               # v - vjx
            tt(g2, lya, g1, Alu.mult)
            tt(g3, lxa, vjy, Alu.mult)
            tt(g2, g2, g3, Alu.add)                      # Gw/2 = ly(v-vjx)+lx vjy
            stt(a2[:, :, 16:24], g2, 2.0, Alu.mult, aa, Alu.mult)
            stt(a2[:, :, 24:32], g2, -2.0, Alu.mult, aa, Alu.mult)
            # SP = 2v^2 - 4 v vjx + 2(vjx^2+vjy^2) - 4 lx v + 4 lx vjx + 4 ly vjy + lx^2 + ly^2
            tt(g1, vjx, vjx, Alu.mult)
            tt(g2, vjy, vjy, Alu.mult)
            tt(g1, g1, g2, Alu.add)                      # vjx^2+vjy^2
            tt(g2, lxa, lxa, Alu.mult)
            tt(g3, lya, lya, Alu.mult)
            tt(g2, g2, g3, Alu.add)                      # lx^2+ly^2
            stt(g4, g1, 2.0, Alu.mult, g2, Alu.add)      # acc
            tt(g1, bv8, vjx, Alu.mult)
            stt(g4, g1, -4.0, Alu.mult, g4, Alu.add)
            tt(g1, lxa, bv8, Alu.mult)
            stt(g4, g1, -4.0, Alu.mult, g4, Alu.add)
            tt(g1, lxa, vjx, Alu.mult)
            stt(g4, g1, 4.0, Alu.mult, g4, Alu.add)
            tt(g1, lya, vjy, Alu.mult)
            stt(g4, g1, 4.0, Alu.mult, g4, Alu.add)
            ts(s2, s1, 2.0, Alu.mult)                    # 2v^2
            tt(g4, g4, bc(s2, _NA), Alu.add)             # SP
            stt(g5, g4, -0.25, Alu.add, aa, Alu.mult)
            vec.tensor_copy(out=b[:, :, 16:24], in_=g5)
            ts(g5, g4, -1.0, Alu.mult, 100.0, Alu.add)
            tt(b[:, :, 24:32], g5, aa, Alu.mult)

            # box rows 32:37
            vec.memset(a1[:, :, 32:37], 0.0)
            vec.memset(a2[:, :, 32:37], 0.0)
            vec.memset(a1[:, :, 32], -1.0)
            vec.memset(a1[:, :, 33], 1.0)
            vec.memset(a2[:, :, 34], -1.0)
            vec.memset(a2[:, :, 35], 1.0)
            vec.memset(b[:, :, 32:36], 1.0)
            vec.memset(b[:, :, 36], 0.0)

            # ---------------- M = Q + A^T A, Minv, B3, c ----------------
            w37 = m1[:, :, 0:_M]
            tt(w37, a1, a1, Alu.mult)
            vec.reduce_sum(out=Mv[0], in_=w37, axis=mybir.AxisListType.X)   # sum a1^2 (box adds 2)
            tt(w37, a1, a2, Alu.mult)
            vec.reduce_sum(out=Mv[1], in_=w37, axis=mybir.AxisListType.X)   # M12
            tt(w37, a2, a2, Alu.mult)
            vec.reduce_sum(out=Mv[3], in_=w37, axis=mybir.AxisListType.X)
            vec.reduce_sum(out=s1, in_=a1[:, :, 0:_NO], axis=mybir.AxisListType.X)
            ts(Mv[2], s1, -1.0, Alu.mult)                                   # M13
            vec.reduce_sum(out=s1, in_=a2[:, :, 0:_NO], axis=mybir.AxisListType.X)
            ts(Mv[4], s1, -1.0, Alu.mult)                                   # M23
            ts(Mv[0], Mv[0], 2.0, Alu.add)                                  # M11
            ts(Mv[3], Mv[3], 2.0, Alu.add)                                  # M22
            M11, M12, M13, M22, M23 = Mv
            # cofactors (M33 const)
            tt(s1, M23, M23, Alu.mult)
            stt(Cf[0], M22, _M33, Alu.mult, s1, Alu.subtract)               # c11
            tt(s1, M13, M23, Alu.mult)
            stt(Cf[1], M12, -_M33, Alu.mult, s1, Alu.add)                   # c12
            tt(s1, M12, M23, Alu.mult)
            tt(s2, M13, M22, Alu.mult)
            tt(Cf[2], s1, s2, Alu.subtract)                                 # c13
            tt(s1, M13, M13, Alu.mult)
            stt(Cf[3], M11, _M33, Alu.mult, s1, Alu.subtract)               # c22
            tt(s1, M12, M13, Alu.mult)
            tt(s2, M11, M23, Alu.mult)
            tt(Cf[4], s1, s2, Alu.subtract)                                 # c23
            tt(s1, M11, M22, Alu.mult)
            tt(s2, M12, M12, Alu.mult)
            tt(Cf[5], s1, s2, Alu.subtract)                                 # c33
            # det, 1/det, scale cofactors
            tt(s1, M11, Cf[0], Alu.mult)
            tt(s2, M12, Cf[1], Alu.mult)
            tt(s1, s1, s2, Alu.add)
            tt(s2, M13, Cf[2], Alu.mult)
            tt(s1, s1, s2, Alu.add)
            vec.reciprocal(out=s2, in_=s1)
            for i in range(6):
                tt(Cf[i], Cf[i], s2, Alu.mult)
            # B3_j = Minv_j. @ A^T ; col 37 = c_j = 2(Minv_j1 u1 + Minv_j2 u2)
            rows = [(Cf[0], Cf[1], Cf[2]), (Cf[1], Cf[3], Cf[4]), (Cf[2], Cf[4], Cf[5])]
            u1 = ut[:, :, 0]
            u2 = ut[:, :, 1]
            for j in range(3):
                cj1, cj2, cj3 = rows[j]
                Bj = B3[j][:, :, 0:_M]
                tt(Bj, a1, bc(cj1, _M), Alu.mult)
                tt(w37, a2, bc(cj2, _M), Alu.mult)
                tt(Bj, Bj, w37, Alu.add)
                tt(B3[j][:, :, 0:_NO], B3[j][:, :, 0:_NO], bc(cj3, _NO), Alu.subtract)
                tt(B3[j][:, :, 36], B3[j][:, :, 36], cj3, Alu.subtract)
                tt(s1, cj1, u1, Alu.mult)
                tt(s2, cj2, u2, Alu.mult)
                tt(s1, s1, s2, Alu.add)
                ts(B3[j][:, :, 37], s1, 2.0, Alu.mult)

            # ---------------- ADMM state init ----------------
            vec.memset(text[:, :, 37], 1.0)
            vec.tensor_scalar_min(out=text[:, :, 0:_M], in0=b, scalar1=0.0)  # t0 = min(0, b)
            vec.memset(y[:], 0.0)

            # ---------------- 100 ADMM iterations ----------------
            # relu form: w = v + y; y' = relu(w - b) (ACT); t = w - 2y'
            segop = _register_segsum_op() if segsum else None
            text_flat = text.rearrange("p c k -> p (c k)")
            mS_flat = [m.rearrange("p c k -> p (c k)") for m in mS]
            halves = (slice(0, _C // 2), slice(_C // 2, _C))
            for it in range(n_iters):
                nj = 2 if it == n_iters - 1 else 3
                if segsum:
                    for j in range(nj):
                        vec._custom_dve(segop, out=mS_flat[j],
                                        in0=B3[j][:], in1=text_flat)
                else:
                    for j in range(nj):
                        tt(mS[j][:], B3[j], text, Alu.mult)
                        vec.reduce_sum(out=(x1, x2, x3)[j], in_=mS[j][:],
                                       axis=mybir.AxisListType.X)
                if it == n_iters - 1:
                    break
                tt(vz[:], a1, bc(x1, _M), Alu.mult)            # v1
                tt(ww[:], a2, bc(x2, _M), Alu.mult)            # v2
                tt(ww[:], vz, ww, Alu.add)                     # s = v1 + v2
                tt(ww[:], ww, y, Alu.add)                      # w = s + y
                tt(ww[:, :, 0:_NO], ww[:, :, 0:_NO], bc(x3, _NO), Alu.subtract)
                tt(ww[:, :, 36], ww[:, :, 36], x3, Alu.subtract)
                for h in halves:
                    tt(vz[:, h, :], ww[:, h, :], b[:, h, :], Alu.subtract)   # e = w - b
                for h in halves:
                    nc.scalar.activation(out=y[:, h, :], in_=vz[:, h, :],
                                         func=mybir.ActivationFunctionType.Relu)
                for h in halves:
                    stt(text[:, h, 0:_M], y[:, h, :], -2.0, Alu.mult,
                        ww[:, h, :], Alu.add)                  # t = w - 2 relu

            # ---------------- output ----------------
            vec.tensor_copy(out=o_t[:, :, 0], in_=x1)
            vec.tensor_copy(out=o_t[:, :, 1], in_=x2)
            nc.sync.dma_start(out=out_ext.rearrange("(p c) k -> p c k", p=_P), in_=o_t[:])

    if split_waits:
        _split_excess_waits(nc, mybir)
    return nc


def _split_excess_waits(nc, mybir):
    """Walrus ISA structs carry a limited number of sync-wait slots (1 for
    STT/CTRL structs, 2 for most compute structs); the Tile scheduler can
    attach more (e.g. the tail drain waits on every DMA queue sem).  Move
    excess waits onto same-engine single-wait NoOps inserted directly
    before the instruction."""
    def limit_for(inst):
        return 1

    for fn in nc.m.functions:
        for blk in fn.blocks:
            il = list(blk.instructions)
            new, changed = [], False
            for inst in il:
                si = inst.sync_info
                lim = limit_for(inst)
                if si is not None and len(si.on_wait) > lim:
                    waits = list(si.on_wait)
                    k = 0
                    while len(waits) > lim:
                        new.append(mybir.InstNoOp(
                            name=f"{inst.name}-waitsplit{k}",
                            ins=[], outs=[], engine=inst.engine,
                            sync_info=mybir.SyncInfo(on_wait=[waits.pop(0)], on_update=[]),
                            bass_nofuse=True,
                        ))
                        k += 1
                    inst.sync_info = mybir.SyncInfo(on_wait=waits, on_update=si.on_update)
                    changed = True
                new.append(inst)
            if changed:
                blk.instructions = new


def _get_program():
    if "nc" not in _cache:
        _cache["nc"] = _build_program()
    return _cache["nc"]


def _run(in_maps, trace=False):
    from concourse.bass_utils import run_bass_kernel_spmd

    nc = _get_program()
    return run_bass_kernel_spmd(nc, in_maps, list(range(_N_CORES)), trace=trace)


def _shard(inputs):
    in_maps = []
    for i in range(_N_CORES):
        sl = slice(i * _BC, (i + 1) * _BC)
        in_maps.append({
            k: np.ascontiguousarray(np.asarray(v)[sl], dtype=np.float32)
            for k, v in inputs.items()
        })
    return in_maps


def kernel(**inputs):
    res = _run(_shard(inputs))
    return np.concatenate([r["out"] for r in res.results], axis=0)
